# revision 1
# baseline (speedup 1.0000x reference)
"""KNN anomaly-scoring kernel for Trainium2 (Bass/Tile), 8 NeuronCores.

Model: for each of B=8 images with a [768, 32, 32] embedding grid, compute the
mean Euclidean distance to the 3 nearest neighbors in a 20000x768 memory bank
per spatial location, then bilinear-upsample the 32x32 score map to 512x512.

Sharding: data-parallel over batch. Core b handles image b (1024 queries) with
a full bank replica. No collectives.

Per-core device program:
  - Distances via one augmented matmul chain on the TensorEngine:
      psum[q, n] = 2*q.b - q2[q] - b2[n] = -d2[q, n]
    with queries stationary ([128,128] fp32r tiles over 768 + 2 aug rows) and
    the bank streamed in 500-column chunks (fp32r => 1 cycle/row at N>=512).
  - Top-3 via the DVE max8 instruction: per-chunk top-8 of -d2 (sorted desc),
    then a final max8 over all 320 candidates per query.
  - score = mean(sqrt(max(d2,1e-12))) over top-3, via ACT Sqrt(scale=-1/9)
    with accum_out.
  - Bilinear 32->512 upsample = R @ S @ R^T, two small matmuls on-device.
"""

import numpy as np

B, E, HL, WL = 8, 768, 32, 32
N_BANK = 20000
Q = HL * WL            # 1024 queries per image
QT = Q // 128          # 8 query tiles
KC = E // 128          # 6 contraction chunks of 128
OUT = 512
NGROUPS = 10           # bank column groups
GW = N_BANK // NGROUPS # 2000 columns per group
CW = 500               # matmul moving-chunk width (one PSUM bank)
NCH = GW // CW         # 4 chunks per group
NCAND = NGROUPS * NCH * 8  # 320 candidate slots per query

_CACHE = {}


def _build_nc():
    import concourse.bass as bass
    import concourse.bacc as bacc
    import concourse.mybir as mybir
    from concourse.tile import TileContext

    f32 = mybir.dt.float32
    f32r = mybir.dt.float32r

    nc = bacc.Bacc("TRN2", target_bir_lowering=False, debug=False)

    qk_d = nc.dram_tensor("qk", [KC, 128, Q], f32r, kind="ExternalInput")
    qaug_d = nc.dram_tensor("qaug", [2, Q], f32r, kind="ExternalInput")
    bankt_d = nc.dram_tensor("bankt", [NGROUPS, KC, 128, GW], f32r, kind="ExternalInput")
    baug_d = nc.dram_tensor("baug", [2, N_BANK], f32r, kind="ExternalInput")
    rt_d = nc.dram_tensor("rt", [32, OUT], f32, kind="ExternalInput")
    out_d = nc.dram_tensor("out", [OUT, OUT], f32, kind="ExternalOutput")

    with TileContext(nc) as tc:
        with (
            tc.tile_pool(name="qpool", bufs=1) as qpool,
            tc.tile_pool(name="bpool", bufs=2) as bpool,
            tc.tile_pool(name="cpool", bufs=1) as cpool,
            tc.tile_pool(name="spool", bufs=2) as spool,
            tc.tile_pool(name="ppool", bufs=8, space=bass.MemorySpace.PSUM) as ppool,
            tc.tile_pool(name="dpool", bufs=1, space=bass.MemorySpace.DRAM) as dpool,
        ):
            # ---- persistent loads ----
            qk_sb = []
            for k in range(KC):
                t_ = qpool.tile([128, Q], f32r, tag=f"qk{k}")
                nc.sync.dma_start(t_[:], qk_d[k])
                qk_sb.append(t_)
            qaug_sb = qpool.tile([2, Q], f32r, tag="qaug")
            nc.sync.dma_start(qaug_sb[:], qaug_d[:])
            rt_sb = qpool.tile([32, OUT], f32, tag="rt")
            nc.sync.dma_start(rt_sb[:], rt_d[:])

            cand = [
                cpool.tile([128, NCAND], f32, tag=f"cand{t}", name=f"cand{t}")
                for t in range(QT)
            ]
            scores_sb = cpool.tile([128, QT], f32, tag="scores")

            # ---- distance + per-chunk top-8 ----
            for g in range(NGROUPS):
                bk = bpool.tile([128, KC, GW], f32r, tag="bank")
                for k in range(KC):
                    nc.sync.dma_start(bk[:, k, :], bankt_d[g, k])
                ba = bpool.tile([2, GW], f32r, tag="baug")
                nc.sync.dma_start(ba[:], baug_d[:, g * GW:(g + 1) * GW])
                for t in range(QT):
                    for c in range(NCH):
                        ps = ppool.tile([128, 512], f32, tag="ps")
                        for k in range(KC):
                            nc.tensor.matmul(
                                ps[:, :CW],
                                qk_sb[k][:, t * 128:(t + 1) * 128],
                                bk[:, k, c * CW:(c + 1) * CW],
                                start=(k == 0), stop=False,
                            )
                        nc.tensor.matmul(
                            ps[:, :CW],
                            qaug_sb[:, t * 128:(t + 1) * 128],
                            ba[:, c * CW:(c + 1) * CW],
                            start=False, stop=True,
                        )
                        nc.vector.max(
                            cand[t][:, (g * NCH + c) * 8:(g * NCH + c + 1) * 8],
                            ps[:, :CW],
                        )

            # ---- final top-3 -> mean distance per query ----
            for t in range(QT):
                fin8 = spool.tile([128, 8], f32, tag="fin8")
                nc.vector.max(fin8[:], cand[t][:])
                v3 = spool.tile([128, 3], f32, tag="v3")
                nc.vector.tensor_scalar_min(v3[:], fin8[:, 0:3], -1e-12)
                d3 = spool.tile([128, 3], f32, tag="d3")
                nc.scalar.activation(
                    d3[:], v3[:], mybir.ActivationFunctionType.Sqrt,
                    scale=-1.0 / 9.0, accum_out=scores_sb[:, t:t + 1],
                )

            # ---- re-layout scores [128, 8] -> S [32, 32] via DRAM bounce ----
            # query index q = t*128 + p; (h, w) = (q // 32, q % 32)
            dscr = dpool.tile([HL, WL], f32, tag="dscr")
            nc.sync.dma_start(
                dscr[:].rearrange("(t x) w -> (x w) t", t=QT), scores_sb[:]
            )
            st_sb = spool.tile([32, 32], f32, tag="st")  # st[w, h] = S[h, w]
            nc.sync.dma_start(st_sb[:], dscr[:].rearrange("h w -> w h"))

            # ---- bilinear upsample: out = R @ S @ R^T ----
            psu = ppool.tile([128, 512], f32, tag="ps")
            nc.tensor.matmul(psu[:32, :], st_sb[:], rt_sb[:],
                             start=True, stop=True)
            u_sb = spool.tile([32, OUT], f32, tag="u")
            nc.vector.tensor_copy(u_sb[:], psu[:32, :])
            for i in range(4):
                po = ppool.tile([128, 512], f32, tag="ps")
                nc.tensor.matmul(po[:], rt_sb[:, i * 128:(i + 1) * 128],
                                 u_sb[:], start=True, stop=True)
                osb = spool.tile([128, OUT], f32, tag="osb")
                nc.vector.tensor_copy(osb[:], po[:])
                nc.sync.dma_start(out_d[i * 128:(i + 1) * 128, :], osb[:])

    nc.compile()
    return nc


def _resize_matrix(n_in: int, n_out: int) -> np.ndarray:
    """Bilinear (half-pixel, edge-clamped) interpolation matrix [n_out, n_in].
    Matches jax.image.resize(method='bilinear') for upsampling."""
    R = np.zeros((n_out, n_in), dtype=np.float64)
    scale = n_in / n_out
    for i in range(n_out):
        src = (i + 0.5) * scale - 0.5
        a0 = int(np.floor(src))
        w = src - a0
        a0c = min(max(a0, 0), n_in - 1)
        a1c = min(max(a0 + 1, 0), n_in - 1)
        R[i, a0c] += 1.0 - w
        R[i, a1c] += w
    return R.astype(np.float32)


def _prep_inputs(embeddings: np.ndarray, bank: np.ndarray):
    """Host-side layout prep. Returns per-core input maps."""
    f = np.float32
    emb = np.asarray(embeddings, dtype=f)
    bank = np.asarray(bank, dtype=f)

    # queries: [B, E, HL, WL] -> qT [B, E, Q] (E-major for the stationary side)
    qT = emb.reshape(B, E, Q)
    q2 = np.einsum("beq,beq->bq", qT, qT)              # [B, Q]
    qk_all = np.ascontiguousarray(qT.reshape(B, KC, 128, Q))
    qaug_all = np.stack(
        [q2, np.ones((B, Q), dtype=f)], axis=1
    ).astype(f)                                         # [B, 2, Q]

    bankT2 = np.ascontiguousarray((2.0 * bank).T)       # [E, N]
    bankt = np.ascontiguousarray(
        bankT2.reshape(KC, 128, NGROUPS, GW).transpose(2, 0, 1, 3)
    )                                                   # [NGROUPS, KC, 128, GW]
    b2 = np.einsum("ne,ne->n", bank, bank)
    baug = np.stack([-np.ones(N_BANK, dtype=f), -b2]).astype(f)  # [2, N]

    rt = np.ascontiguousarray(_resize_matrix(HL, OUT).T)  # [32, 512]

    in_maps = [
        {
            "qk": qk_all[b],
            "qaug": np.ascontiguousarray(qaug_all[b]),
            "bankt": bankt,
            "baug": baug,
            "rt": rt,
        }
        for b in range(B)
    ]
    return in_maps


def kernel(embeddings, bank, out_size, _trace=False, _trace_kwargs=None):
    from concourse import bass_utils

    assert int(out_size) == OUT
    if "nc" not in _CACHE:
        _CACHE["nc"] = _build_nc()
    nc = _CACHE["nc"]

    in_maps = _prep_inputs(np.asarray(embeddings), np.asarray(bank))
    res = bass_utils.run_bass_kernel_spmd(
        nc, in_maps, core_ids=list(range(B)), trace=_trace,
        **(_trace_kwargs or {}),
    )
    _CACHE["last_results"] = res
    out = np.stack([res.results[b]["out"] for b in range(B)])
    return out.reshape(B, 1, OUT, OUT).astype(np.float32)



# revision 8
# speedup vs baseline: 2.1951x; 2.1951x over previous
"""KNN anomaly-scoring kernel for Trainium2 (Bass/Tile), 8 NeuronCores.

Model: for each of B=8 images with a [768, 32, 32] embedding grid, compute the
mean Euclidean distance to the 3 nearest neighbors in a 20000x768 memory bank
per spatial location, then bilinear-upsample the 32x32 score map to 512x512.

Sharding: data-parallel over batch. Core b handles image b (1024 queries) with
a full bank replica. No collectives.

Per-core device program (v2 — fp8 DoubleRow + 4-engine top-k scan):
  - Ranking key on the TensorEngine in fp8e4 DoubleRow mode (2 k-tiles of 128
    per instruction, 0.5 cycles/output column):
      r[q, n] = 2q.b - (b2[n] - 768)
    via 3 DoubleRow matmuls over E=768 plus one k=1 DoubleRow "aug" matmul
    whose two pairs carry (b2-768) as a two-term fp8 expansion (coarse +
    residual), keeping b2 nearly exact. d2 = (q2 + 768) - r with q2 applied
    later as a per-partition activation bias.
  - Bank padded 20000 -> 20480 (pad entries get r = -448, never selected),
    10 groups of 2048 columns; PSUM as two [128, 2048] 4-bank tiles.
  - Top-k scan split across the three PSUM-capable/SBUF engines per chunk:
    ACT copies cols [0:1664] PSUM->SBUF, Pool folds them 2->1 with
    tensor_tensor(max), DVE max8's the folded 832 plus the remaining 384
    PSUM cols directly. 16 candidates per (query tile, group).
  - Final: DVE max8 over 160 candidates, ACT Sqrt(r*(-1/9) + (q2+768)/9)
    with accum_out -> mean of 3 neighbor distances.
  - Bilinear 32->512 upsample = R @ S @ R^T, two small f32r matmuls.
"""

import numpy as np

B, E, HL, WL = 8, 768, 32, 32
N_BANK = 20000
Q = HL * WL            # 1024 queries per image
QT = Q // 128          # 8 query tiles
KC = E // 128          # 6 contraction k-tiles of 128
KK = KC // 2           # 3 DoubleRow pairs
OUT = 512
NPAD = 20480
NG = 10                # bank column groups
GW = NPAD // NG        # 2048 columns per group
SC = GW // 512         # 4 matmul sub-chunks per group (one PSUM bank each)
WA = 1760              # ACT copy width per chunk (DVE folds 4->1 in bf16)
WTAIL = GW - WA        # DVE direct-psum max8 width (288)
NCAND = NG * 16        # 160 candidate slots per query

_CACHE = {}


def _build_nc():
    import concourse.bass as bass
    import concourse.bacc as bacc
    import concourse.mybir as mybir
    from concourse.tile import TileContext

    f32 = mybir.dt.float32
    f32r = mybir.dt.float32r
    f8 = mybir.dt.float8e4
    bf16 = mybir.dt.bfloat16
    DR = mybir.MatmulPerfMode.DoubleRow

    nc = bacc.Bacc("TRN2", target_bir_lowering=False, debug=False)

    qk8_d = nc.dram_tensor("qk8", [KC, 128, Q], f8, kind="ExternalInput")
    qb_d = nc.dram_tensor("qb", [128, QT], f32, kind="ExternalInput")
    bank8_d = nc.dram_tensor("bank8", [NG, KC, 128, GW], f8, kind="ExternalInput")
    aug_d = nc.dram_tensor("aug", [NG, 2, GW], f8, kind="ExternalInput")
    rt_d = nc.dram_tensor("rt", [32, OUT], f32, kind="ExternalInput")
    out_d = nc.dram_tensor("out", [OUT, OUT], f32, kind="ExternalOutput")

    with TileContext(nc) as tc:
        with (
            tc.tile_pool(name="qpool", bufs=1) as qpool,
            tc.tile_pool(name="bpool", bufs=2) as bpool,
            tc.tile_pool(name="cpool", bufs=1) as cpool,
            tc.tile_pool(name="spool", bufs=2) as spool,
            tc.tile_pool(name="fpool", bufs=2) as fpool,
            tc.tile_pool(name="ppool", bufs=2, space=bass.MemorySpace.PSUM) as ppool,
            tc.tile_pool(name="dpool", bufs=1, space=bass.MemorySpace.DRAM) as dpool,
        ):
            # ---- persistent loads ----
            qk_sb = qpool.tile([128, KC, Q], f8, tag="qk")
            for k in range(KC):
                nc.sync.dma_start(qk_sb[:, k, :], qk8_d[k])
            qb_sb = qpool.tile([128, QT], f32, tag="qb")
            nc.sync.dma_start(qb_sb[:], qb_d[:])
            rt_sb = qpool.tile([32, OUT], f32, tag="rt")
            nc.sync.dma_start(rt_sb[:], rt_d[:])
            neg1_sb = qpool.tile([1, 2, 128], f8, tag="neg1")
            nc.vector.memset(neg1_sb[:], -1.0)

            cand = [
                cpool.tile([128, NCAND], f32, tag=f"cand{t}", name=f"cand{t}")
                for t in range(QT)
            ]
            scores_sb = cpool.tile([128, QT], f32, tag="scores")

            # ---- ranking key + per-chunk top-16 ----
            for g in range(NG):
                bk = bpool.tile([128, KC, GW], f8, tag="bank")
                for k in range(KC):
                    nc.sync.dma_start(bk[:, k, :], bank8_d[g, k])
                au = bpool.tile([1, 2, GW], f8, tag="aug")
                nc.sync.dma_start(au[:], aug_d[g])
                for t in range(QT):
                    ps = ppool.tile([128, GW], f32, tag="mega")
                    for c in range(SC):
                        dst = ps[:, c * 512:(c + 1) * 512]
                        for kk in range(KK):
                            nc.tensor.matmul(
                                dst,
                                qk_sb[:, 2 * kk:2 * kk + 2, t * 128:(t + 1) * 128],
                                bk[:, 2 * kk:2 * kk + 2, c * 512:(c + 1) * 512],
                                start=(kk == 0), stop=False, perf_mode=DR,
                            )
                        nc.tensor.matmul(
                            dst,
                            neg1_sb[:],
                            au[:, :, c * 512:(c + 1) * 512],
                            start=False, stop=True, perf_mode=DR,
                        )
                    cp = spool.tile([128, WA], bf16, tag="cp")
                    nc.scalar.copy(cp[:], ps[:, 0:WA])
                    f1 = spool.tile([128, WA // 2], bf16, tag="f1")
                    nc.vector.tensor_tensor(
                        f1[:], cp[:, 0:WA // 2], cp[:, WA // 2:WA],
                        op=mybir.AluOpType.max,
                    )
                    f2 = spool.tile([128, WA // 4], bf16, tag="f2")
                    nc.vector.tensor_tensor(
                        f2[:], f1[:, 0:WA // 4], f1[:, WA // 4:WA // 2],
                        op=mybir.AluOpType.max,
                    )
                    nc.vector.max(cand[t][:, g * 16:g * 16 + 8], f2[:])
                    nc.vector.max(cand[t][:, g * 16 + 8:g * 16 + 16],
                                  ps[:, WA:GW])

            # ---- final top-3 -> mean distance per query ----
            for t in range(QT):
                fin8 = fpool.tile([128, 8], f32, tag="fin8")
                nc.vector.max(fin8[:], cand[t][:])
                d3 = fpool.tile([128, 3], f32, tag="d3")
                nc.scalar.activation(
                    d3[:], fin8[:, 0:3], mybir.ActivationFunctionType.Sqrt,
                    scale=-1.0 / 9.0, bias=qb_sb[:, t:t + 1],
                    accum_out=scores_sb[:, t:t + 1],
                )

            # ---- re-layout scores [128, 8] -> S [32, 32] via DRAM bounce ----
            # query index q = t*128 + p; (h, w) = (q // 32, q % 32)
            dscr = dpool.tile([HL, WL], f32, tag="dscr")
            nc.sync.dma_start(
                dscr[:].rearrange("(t x) w -> (x w) t", t=QT), scores_sb[:]
            )
            st_sb = fpool.tile([32, 32], f32, tag="st")  # st[w, h] = S[h, w]
            nc.sync.dma_start(st_sb[:], dscr[:].rearrange("h w -> w h"))

            # ---- bilinear upsample: out = R @ S @ R^T ----
            psu = ppool.tile([128, GW], f32, tag="mega")
            nc.tensor.matmul(psu[:32, :OUT], st_sb[:], rt_sb[:],
                             start=True, stop=True)
            u_sb = fpool.tile([32, OUT], f32, tag="u")
            nc.vector.tensor_copy(u_sb[:], psu[:32, :OUT])
            for i in range(4):
                po = ppool.tile([128, GW], f32, tag="mega")
                nc.tensor.matmul(po[:, :OUT], rt_sb[:, i * 128:(i + 1) * 128],
                                 u_sb[:], start=True, stop=True)
                osb = fpool.tile([128, OUT], f32, tag="osb")
                nc.vector.tensor_copy(osb[:], po[:, :OUT])
                nc.sync.dma_start(out_d[i * 128:(i + 1) * 128, :], osb[:])

    nc.compile()
    return nc


def _resize_matrix(n_in: int, n_out: int) -> np.ndarray:
    """Bilinear (half-pixel, edge-clamped) interpolation matrix [n_out, n_in].
    Matches jax.image.resize(method='bilinear') for upsampling."""
    R = np.zeros((n_out, n_in), dtype=np.float64)
    scale = n_in / n_out
    for i in range(n_out):
        src = (i + 0.5) * scale - 0.5
        a0 = int(np.floor(src))
        w = src - a0
        a0c = min(max(a0, 0), n_in - 1)
        a1c = min(max(a0 + 1, 0), n_in - 1)
        R[i, a0c] += 1.0 - w
        R[i, a1c] += w
    return R.astype(np.float32)


def _prep_inputs(embeddings: np.ndarray, bank: np.ndarray):
    """Host-side layout prep. Returns per-core input maps."""
    import ml_dtypes
    f = np.float32
    f8 = ml_dtypes.float8_e4m3fn
    emb = np.asarray(embeddings, dtype=f)
    bank = np.asarray(bank, dtype=f)

    # queries: [B, E, HL, WL] -> qT [B, E, Q] (E-major for the stationary side)
    qT = emb.reshape(B, E, Q)
    q2 = np.einsum("beq,beq->bq", qT, qT)               # [B, Q]
    qk8_all = (2.0 * qT).astype(f8).reshape(B, KC, 128, Q)
    qb_all = ((q2 + 768.0) / 9.0).reshape(B, QT, 128).transpose(0, 2, 1)

    bankP = np.zeros((NPAD, E), dtype=f)
    bankP[:N_BANK] = bank
    bank8 = np.ascontiguousarray(
        bankP.T.reshape(KC, 128, NG, GW).transpose(2, 0, 1, 3).astype(f8)
    )                                                   # [NG, KC, 128, GW]
    b2c = np.full(NPAD, np.nan, dtype=f)
    b2c[:N_BANK] = np.einsum("ne,ne->n", bank, bank) - 768.0
    c0 = b2c.astype(f8)
    c1 = (b2c - c0.astype(f)).astype(f8)
    c0[N_BANK:] = f8(224.0)
    c1[N_BANK:] = f8(224.0)
    aug = np.ascontiguousarray(
        np.stack([c0, c1], axis=0).reshape(2, NG, GW).transpose(1, 0, 2)
    )                                                   # [NG, 2, GW]

    rt = np.ascontiguousarray(_resize_matrix(HL, OUT).T)  # [32, 512]

    in_maps = [
        {
            "qk8": np.ascontiguousarray(qk8_all[b]),
            "qb": np.ascontiguousarray(qb_all[b].astype(f)),
            "bank8": bank8,
            "aug": aug,
            "rt": rt,
        }
        for b in range(B)
    ]
    return in_maps


def kernel(embeddings, bank, out_size, _trace=False, _trace_kwargs=None):
    from concourse import bass_utils

    assert int(out_size) == OUT
    if "nc" not in _CACHE:
        _CACHE["nc"] = _build_nc()
    nc = _CACHE["nc"]

    in_maps = _prep_inputs(np.asarray(embeddings), np.asarray(bank))
    res = bass_utils.run_bass_kernel_spmd(
        nc, in_maps, core_ids=list(range(B)), trace=_trace,
        **(_trace_kwargs or {}),
    )
    _CACHE["last_results"] = res
    out = np.stack([res.results[b]["out"] for b in range(B)])
    return out.reshape(B, 1, OUT, OUT).astype(np.float32)


# revision 12
# speedup vs baseline: 2.3968x; 1.0919x over previous
"""KNN anomaly-scoring kernel for Trainium2 (Bass/Tile), 8 NeuronCores.

Model: for each of B=8 images with a [768, 32, 32] embedding grid, compute the
mean Euclidean distance to the 3 nearest neighbors in a 20000x768 memory bank
per spatial location, then bilinear-upsample the 32x32 score map to 512x512.

Sharding: data-parallel over batch. Core b handles image b (1024 queries) with
a full bank replica. No collectives.

Per-core device program (v2 — fp8 DoubleRow + 4-engine top-k scan):
  - Ranking key on the TensorEngine in fp8e4 DoubleRow mode (2 k-tiles of 128
    per instruction, 0.5 cycles/output column):
      r[q, n] = 2q.b - (b2[n] - 768)
    via 3 DoubleRow matmuls over E=768 plus one k=1 DoubleRow "aug" matmul
    whose two pairs carry (b2-768) as a two-term fp8 expansion (coarse +
    residual), keeping b2 nearly exact. d2 = (q2 + 768) - r with q2 applied
    later as a per-partition activation bias.
  - Bank padded 20000 -> 20480 (pad entries get r = -448, never selected),
    10 groups of 2048 columns; PSUM as two [128, 2048] 4-bank tiles.
  - Top-k scan split across the three PSUM-capable/SBUF engines per chunk:
    ACT copies cols [0:1664] PSUM->SBUF, Pool folds them 2->1 with
    tensor_tensor(max), DVE max8's the folded 832 plus the remaining 384
    PSUM cols directly. 16 candidates per (query tile, group).
  - Final: DVE max8 over 160 candidates, ACT Sqrt(r*(-1/9) + (q2+768)/9)
    with accum_out -> mean of 3 neighbor distances.
  - Bilinear 32->512 upsample = R @ S @ R^T, two small f32r matmuls.
"""

import numpy as np

B, E, HL, WL = 8, 768, 32, 32
N_BANK = 20000
Q = HL * WL            # 1024 queries per image
QT = Q // 128          # 8 query tiles
KC = E // 128          # 6 contraction k-tiles of 128
KK = KC // 2           # 3 DoubleRow pairs
OUT = 512
NPAD = 20480
NG = 10                # bank column groups
GW = NPAD // NG        # 2048 columns per group
SC = GW // 512         # 4 matmul sub-chunks per group (one PSUM bank each)
WA = 1728              # ACT copy width per chunk (DVE folds 8->1 in bf16)
WTAIL = GW - WA        # DVE direct-psum max8 width (320)
NCAND = NG * 16        # 160 candidate slots per query

_CACHE = {}


def _build_nc():
    import concourse.bass as bass
    import concourse.bacc as bacc
    import concourse.mybir as mybir
    from concourse.tile import TileContext

    f32 = mybir.dt.float32
    f32r = mybir.dt.float32r
    f8 = mybir.dt.float8e4
    bf16 = mybir.dt.bfloat16
    DR = mybir.MatmulPerfMode.DoubleRow

    nc = bacc.Bacc("TRN2", target_bir_lowering=False, debug=False)

    qk8_d = nc.dram_tensor("qk8", [KC, 128, Q], f8, kind="ExternalInput")
    qb_d = nc.dram_tensor("qb", [128, QT], f32, kind="ExternalInput")
    bank8_d = nc.dram_tensor("bank8", [NG, KC, 128, GW], f8, kind="ExternalInput")
    aug_d = nc.dram_tensor("aug", [NG, 2, GW], f8, kind="ExternalInput")
    rt_d = nc.dram_tensor("rt", [32, OUT], f32, kind="ExternalInput")
    out_d = nc.dram_tensor("out", [OUT, OUT], f32, kind="ExternalOutput")

    with TileContext(nc) as tc:
        with (
            tc.tile_pool(name="qpool", bufs=1) as qpool,
            tc.tile_pool(name="bpool", bufs=2) as bpool,
            tc.tile_pool(name="cpool", bufs=1) as cpool,
            tc.tile_pool(name="spool", bufs=3) as spool,
            tc.tile_pool(name="fpool", bufs=2) as fpool,
            tc.tile_pool(name="ppool", bufs=2, space=bass.MemorySpace.PSUM) as ppool,
            tc.tile_pool(name="dpool", bufs=1, space=bass.MemorySpace.DRAM) as dpool,
        ):
            # ---- persistent loads (per-k tiles for fine-grained DMA deps) ----
            qk_sb = [qpool.tile([128, 2, Q], f8, tag=f"qk{kk}", name=f"qk{kk}")
                     for kk in range(KK)]
            for kk in range(KK):
                for j in range(2):
                    nc.sync.dma_start(qk_sb[kk][:, j, :], qk8_d[2 * kk + j])
            qb_sb = qpool.tile([128, QT], f32, tag="qb")
            nc.sync.dma_start(qb_sb[:], qb_d[:])
            rt_sb = qpool.tile([32, OUT], f32, tag="rt")
            nc.sync.dma_start(rt_sb[:], rt_d[:])
            neg1_sb = qpool.tile([1, 2, 128], f8, tag="neg1")
            nc.vector.memset(neg1_sb[:], -1.0)

            cand = [
                cpool.tile([128, NCAND], f32, tag=f"cand{t}", name=f"cand{t}")
                for t in range(QT)
            ]
            scores_sb = cpool.tile([128, QT], f32, tag="scores")

            def emit_final(t):
                fin8 = fpool.tile([128, 8], f32, tag="fin8")
                nc.vector.max(fin8[:], cand[t][:])
                d3 = fpool.tile([128, 3], f32, tag="d3")
                nc.scalar.activation(
                    d3[:], fin8[:, 0:3], mybir.ActivationFunctionType.Sqrt,
                    scale=-1.0 / 9.0, bias=qb_sb[:, t:t + 1],
                    accum_out=scores_sb[:, t:t + 1],
                )

            def emit_folds(cp, g, t):
                # bf16 fold chain 1728 -> 864 -> 432 -> 216, then top-8.
                f1 = spool.tile([128, WA // 2], bf16, tag="f1")
                nc.vector.tensor_tensor(
                    f1[:], cp[:, 0:WA // 2], cp[:, WA // 2:WA],
                    op=mybir.AluOpType.max,
                )
                f2 = spool.tile([128, WA // 4], bf16, tag="f2")
                nc.vector.tensor_tensor(
                    f2[:], f1[:, 0:WA // 4], f1[:, WA // 4:WA // 2],
                    op=mybir.AluOpType.max,
                )
                f3 = spool.tile([128, WA // 8], bf16, tag="f3")
                nc.vector.tensor_tensor(
                    f3[:], f2[:, 0:WA // 8], f2[:, WA // 8:WA // 4],
                    op=mybir.AluOpType.max,
                )
                nc.vector.max(cand[t][:, g * 16:g * 16 + 8], f3[:])
                if g == NG - 1:
                    emit_final(t)

            # ---- ranking key + per-chunk top-16 (1-chunk fold pipeline) ----
            prev = None
            for g in range(NG):
                bk = [bpool.tile([128, 2, GW], f8, tag=f"bank{kk}", name=f"bk{kk}")
                      for kk in range(KK)]
                for kk in range(KK):
                    for j in range(2):
                        nc.sync.dma_start(bk[kk][:, j, :], bank8_d[g, 2 * kk + j])
                au = bpool.tile([1, 2, GW], f8, tag="aug")
                nc.sync.dma_start(au[:], aug_d[g])
                for t in range(QT):
                    ps = ppool.tile([128, GW], f32, tag="mega")
                    for c in range(SC):
                        dst = ps[:, c * 512:(c + 1) * 512]
                        for kk in range(KK):
                            nc.tensor.matmul(
                                dst,
                                qk_sb[kk][:, :, t * 128:(t + 1) * 128],
                                bk[kk][:, :, c * 512:(c + 1) * 512],
                                start=(kk == 0), stop=False, perf_mode=DR,
                            )
                        nc.tensor.matmul(
                            dst,
                            neg1_sb[:],
                            au[:, :, c * 512:(c + 1) * 512],
                            start=False, stop=True, perf_mode=DR,
                        )
                    # PSUM-draining ops first: ACT copy + DVE tail top-8.
                    cp = spool.tile([128, WA], bf16, tag="cp")
                    nc.scalar.copy(cp[:], ps[:, 0:WA])
                    nc.vector.max(cand[t][:, g * 16 + 8:g * 16 + 16],
                                  ps[:, WA:GW])
                    # Fold chain for the PREVIOUS chunk (its copy is done by
                    # now) so DVE never head-of-line blocks on a fresh copy.
                    if prev is not None:
                        emit_folds(*prev)
                    prev = (cp, g, t)
            emit_folds(*prev)

            # ---- re-layout scores [128, 8] -> S [32, 32] via DRAM bounce ----
            # query index q = t*128 + p; (h, w) = (q // 32, q % 32)
            dscr = dpool.tile([HL, WL], f32, tag="dscr")
            nc.sync.dma_start(
                dscr[:].rearrange("(t x) w -> (x w) t", t=QT), scores_sb[:]
            )
            st_sb = fpool.tile([32, 32], f32, tag="st")  # st[w, h] = S[h, w]
            nc.sync.dma_start(st_sb[:], dscr[:].rearrange("h w -> w h"))

            # ---- bilinear upsample: out = R @ S @ R^T ----
            psu = ppool.tile([128, GW], f32, tag="mega")
            nc.tensor.matmul(psu[:32, :OUT], st_sb[:], rt_sb[:],
                             start=True, stop=True)
            u_sb = fpool.tile([32, OUT], f32, tag="u")
            nc.vector.tensor_copy(u_sb[:], psu[:32, :OUT])
            for i in range(4):
                po = ppool.tile([128, GW], f32, tag="mega")
                nc.tensor.matmul(po[:, :OUT], rt_sb[:, i * 128:(i + 1) * 128],
                                 u_sb[:], start=True, stop=True)
                osb = fpool.tile([128, OUT], f32, tag="osb")
                nc.vector.tensor_copy(osb[:], po[:, :OUT])
                nc.sync.dma_start(out_d[i * 128:(i + 1) * 128, :], osb[:])

    nc.compile()
    return nc


def _resize_matrix(n_in: int, n_out: int) -> np.ndarray:
    """Bilinear (half-pixel, edge-clamped) interpolation matrix [n_out, n_in].
    Matches jax.image.resize(method='bilinear') for upsampling."""
    R = np.zeros((n_out, n_in), dtype=np.float64)
    scale = n_in / n_out
    for i in range(n_out):
        src = (i + 0.5) * scale - 0.5
        a0 = int(np.floor(src))
        w = src - a0
        a0c = min(max(a0, 0), n_in - 1)
        a1c = min(max(a0 + 1, 0), n_in - 1)
        R[i, a0c] += 1.0 - w
        R[i, a1c] += w
    return R.astype(np.float32)


def _prep_inputs(embeddings: np.ndarray, bank: np.ndarray):
    """Host-side layout prep. Returns per-core input maps."""
    import ml_dtypes
    f = np.float32
    f8 = ml_dtypes.float8_e4m3fn
    emb = np.asarray(embeddings, dtype=f)
    bank = np.asarray(bank, dtype=f)

    # queries: [B, E, HL, WL] -> qT [B, E, Q] (E-major for the stationary side)
    qT = emb.reshape(B, E, Q)
    q2 = np.einsum("beq,beq->bq", qT, qT)               # [B, Q]
    qk8_all = (2.0 * qT).astype(f8).reshape(B, KC, 128, Q)
    qb_all = ((q2 + 768.0) / 9.0).reshape(B, QT, 128).transpose(0, 2, 1)

    bankP = np.zeros((NPAD, E), dtype=f)
    bankP[:N_BANK] = bank
    bank8 = np.ascontiguousarray(
        bankP.T.reshape(KC, 128, NG, GW).transpose(2, 0, 1, 3).astype(f8)
    )                                                   # [NG, KC, 128, GW]
    b2c = np.full(NPAD, np.nan, dtype=f)
    b2c[:N_BANK] = np.einsum("ne,ne->n", bank, bank) - 768.0
    c0 = b2c.astype(f8)
    c1 = (b2c - c0.astype(f)).astype(f8)
    c0[N_BANK:] = f8(224.0)
    c1[N_BANK:] = f8(224.0)
    aug = np.ascontiguousarray(
        np.stack([c0, c1], axis=0).reshape(2, NG, GW).transpose(1, 0, 2)
    )                                                   # [NG, 2, GW]

    rt = np.ascontiguousarray(_resize_matrix(HL, OUT).T)  # [32, 512]

    in_maps = [
        {
            "qk8": np.ascontiguousarray(qk8_all[b]),
            "qb": np.ascontiguousarray(qb_all[b].astype(f)),
            "bank8": bank8,
            "aug": aug,
            "rt": rt,
        }
        for b in range(B)
    ]
    return in_maps


def kernel(embeddings, bank, out_size, _trace=False, _trace_kwargs=None):
    from concourse import bass_utils

    assert int(out_size) == OUT
    if "nc" not in _CACHE:
        _CACHE["nc"] = _build_nc()
    nc = _CACHE["nc"]

    in_maps = _prep_inputs(np.asarray(embeddings), np.asarray(bank))
    res = bass_utils.run_bass_kernel_spmd(
        nc, in_maps, core_ids=list(range(B)), trace=_trace,
        **(_trace_kwargs or {}),
    )
    _CACHE["last_results"] = res
    out = np.stack([res.results[b]["out"] for b in range(B)])
    return out.reshape(B, 1, OUT, OUT).astype(np.float32)


# revision 14
# speedup vs baseline: 2.4637x; 1.0279x over previous
"""KNN anomaly-scoring kernel for Trainium2 (Bass/Tile), 8 NeuronCores.

Model: for each of B=8 images with a [768, 32, 32] embedding grid, compute the
mean Euclidean distance to the 3 nearest neighbors in a 20000x768 memory bank
per spatial location, then bilinear-upsample the 32x32 score map to 512x512.

Sharding: data-parallel over batch. Core b handles image b (1024 queries) with
a full bank replica. No collectives.

Per-core device program (v2 — fp8 DoubleRow + 4-engine top-k scan):
  - Ranking key on the TensorEngine in fp8e4 DoubleRow mode (2 k-tiles of 128
    per instruction, 0.5 cycles/output column):
      r[q, n] = 2q.b - (b2[n] - 768)
    via 3 DoubleRow matmuls over E=768 plus one k=1 DoubleRow "aug" matmul
    whose two pairs carry (b2-768) as a two-term fp8 expansion (coarse +
    residual), keeping b2 nearly exact. d2 = (q2 + 768) - r with q2 applied
    later as a per-partition activation bias.
  - Bank padded 20000 -> 20480 (pad entries get r = -448, never selected),
    10 groups of 2048 columns; PSUM as two [128, 2048] 4-bank tiles.
  - Top-k scan split across the three PSUM-capable/SBUF engines per chunk:
    ACT copies cols [0:1664] PSUM->SBUF, Pool folds them 2->1 with
    tensor_tensor(max), DVE max8's the folded 832 plus the remaining 384
    PSUM cols directly. 16 candidates per (query tile, group).
  - Final: DVE max8 over 160 candidates, ACT Sqrt(r*(-1/9) + (q2+768)/9)
    with accum_out -> mean of 3 neighbor distances.
  - Bilinear 32->512 upsample = R @ S @ R^T, two small f32r matmuls.
"""

import numpy as np

B, E, HL, WL = 8, 768, 32, 32
N_BANK = 20000
Q = HL * WL            # 1024 queries per image
QT = Q // 128          # 8 query tiles
KC = E // 128          # 6 contraction k-tiles of 128
KK = KC // 2           # 3 DoubleRow pairs
OUT = 512
NPAD = 20480
NG = 10                # bank column groups
GW = NPAD // NG        # 2048 columns per group
SC = GW // 512         # 4 matmul sub-chunks per group (one PSUM bank each)
WA = 1792              # ACT copy width per chunk (DVE folds 8->1 in bf16)
WTAIL = GW - WA        # DVE direct-psum max8 width (256)
NCAND = NG * 16        # 160 candidate slots per query

_CACHE = {}


def _build_nc():
    import concourse.bass as bass
    import concourse.bacc as bacc
    import concourse.mybir as mybir
    from concourse.tile import TileContext

    f32 = mybir.dt.float32
    f32r = mybir.dt.float32r
    f8 = mybir.dt.float8e4
    bf16 = mybir.dt.bfloat16
    DR = mybir.MatmulPerfMode.DoubleRow

    nc = bacc.Bacc("TRN2", target_bir_lowering=False, debug=False)

    qk8_d = nc.dram_tensor("qk8", [KC, 128, Q], f8, kind="ExternalInput")
    qb_d = nc.dram_tensor("qb", [128, QT], f32, kind="ExternalInput")
    bank8_d = nc.dram_tensor("bank8", [NG, KC, 128, GW], f8, kind="ExternalInput")
    aug_d = nc.dram_tensor("aug", [NG, 2, GW], f8, kind="ExternalInput")
    rt_d = nc.dram_tensor("rt", [32, OUT], bf16, kind="ExternalInput")
    out_d = nc.dram_tensor("out", [OUT, OUT], f32, kind="ExternalOutput")

    with TileContext(nc) as tc:
        with (
            tc.tile_pool(name="qpool", bufs=1) as qpool,
            tc.tile_pool(name="bpool", bufs=2) as bpool,
            tc.tile_pool(name="cpool", bufs=1) as cpool,
            tc.tile_pool(name="spool", bufs=3) as spool,
            tc.tile_pool(name="fpool", bufs=2) as fpool,
            tc.tile_pool(name="ppool", bufs=2, space=bass.MemorySpace.PSUM) as ppool,
            tc.tile_pool(name="dpool", bufs=1, space=bass.MemorySpace.DRAM) as dpool,
        ):
            # ---- persistent tiles; DMAs ordered so chunk 0 starts early ----
            qk_sb = [qpool.tile([128, 2, Q], f8, tag=f"qk{kk}", name=f"qk{kk}")
                     for kk in range(KK)]
            for j in range(2):
                nc.sync.dma_start(qk_sb[0][:, j, :], qk8_d[j])
            qb_sb = qpool.tile([128, QT], f32, tag="qb")
            rt_sb = qpool.tile([32, OUT], bf16, tag="rt")
            neg1_sb = qpool.tile([1, 2, 128], f8, tag="neg1")
            nc.vector.memset(neg1_sb[:], -1.0)

            cand = [
                cpool.tile([128, NCAND], f32, tag=f"cand{t}", name=f"cand{t}")
                for t in range(QT)
            ]
            scores_sb = cpool.tile([128, 32], f32, tag="scores")
            nc.vector.memset(scores_sb[:], 0.0)

            def emit_final(t):
                fin8 = fpool.tile([128, 8], f32, tag="fin8")
                nc.vector.max(fin8[:], cand[t][:])
                d3 = fpool.tile([128, 3], f32, tag="d3")
                nc.scalar.activation(
                    d3[:], fin8[:, 0:3], mybir.ActivationFunctionType.Sqrt,
                    scale=-1.0 / 9.0, bias=qb_sb[:, t:t + 1],
                    accum_out=scores_sb[:, t:t + 1],
                )

            def emit_folds(cp, g, t):
                # bf16 fold chain 1792 -> 896 -> 448 -> 224, then top-8.
                f1 = spool.tile([128, WA // 2], bf16, tag="f1")
                nc.vector.tensor_tensor(
                    f1[:], cp[:, 0:WA // 2], cp[:, WA // 2:WA],
                    op=mybir.AluOpType.max,
                )
                f2 = spool.tile([128, WA // 4], bf16, tag="f2")
                nc.vector.tensor_tensor(
                    f2[:], f1[:, 0:WA // 4], f1[:, WA // 4:WA // 2],
                    op=mybir.AluOpType.max,
                )
                f3 = spool.tile([128, WA // 8], bf16, tag="f3")
                nc.vector.tensor_tensor(
                    f3[:], f2[:, 0:WA // 8], f2[:, WA // 8:WA // 4],
                    op=mybir.AluOpType.max,
                )
                nc.vector.max(cand[t][:, g * 16:g * 16 + 8], f3[:])
                if g == NG - 1:
                    emit_final(t)

            # ---- ranking key + per-chunk top-16 (1-chunk fold pipeline) ----
            prev = None
            for g in range(NG):
                bk = [bpool.tile([128, 2, GW], f8, tag=f"bank{kk}", name=f"bk{kk}")
                      for kk in range(KK)]
                au = bpool.tile([1, 2, GW], f8, tag="aug")
                if g == 0:
                    # interleave so the first sub-chunk's inputs arrive first
                    for j in range(2):
                        nc.sync.dma_start(bk[0][:, j, :], bank8_d[g, j])
                    nc.sync.dma_start(au[:], aug_d[g])
                    for kk in range(1, KK):
                        for j in range(2):
                            nc.sync.dma_start(qk_sb[kk][:, j, :],
                                              qk8_d[2 * kk + j])
                        for j in range(2):
                            nc.sync.dma_start(bk[kk][:, j, :],
                                              bank8_d[g, 2 * kk + j])
                    nc.sync.dma_start(qb_sb[:, 0:QT], qb_d[:])
                    nc.sync.dma_start(rt_sb[:], rt_d[:])
                else:
                    for kk in range(KK):
                        for j in range(2):
                            nc.sync.dma_start(bk[kk][:, j, :],
                                              bank8_d[g, 2 * kk + j])
                    nc.sync.dma_start(au[:], aug_d[g])
                for t in range(QT):
                    ps = ppool.tile([128, GW], f32, tag="mega")
                    for c in range(SC):
                        dst = ps[:, c * 512:(c + 1) * 512]
                        for kk in range(KK):
                            nc.tensor.matmul(
                                dst,
                                qk_sb[kk][:, :, t * 128:(t + 1) * 128],
                                bk[kk][:, :, c * 512:(c + 1) * 512],
                                start=(kk == 0), stop=False, perf_mode=DR,
                            )
                        nc.tensor.matmul(
                            dst,
                            neg1_sb[:],
                            au[:, :, c * 512:(c + 1) * 512],
                            start=False, stop=True, perf_mode=DR,
                        )
                    # PSUM-draining ops first: ACT copy + DVE tail top-8.
                    cp = spool.tile([128, WA], bf16, tag="cp")
                    nc.scalar.copy(cp[:], ps[:, 0:WA])
                    nc.vector.max(cand[t][:, g * 16 + 8:g * 16 + 16],
                                  ps[:, WA:GW])
                    # Fold chain for the PREVIOUS chunk (its copy is done by
                    # now) so DVE never head-of-line blocks on a fresh copy.
                    if prev is not None:
                        emit_folds(*prev)
                    prev = (cp, g, t)
            emit_folds(*prev)

            # ---- re-layout scores [128, 8] -> S [32, 32] via DRAM bounce ----
            # query index q = t*128 + p; (h, w) = (q // 32, q % 32)
            scores_bf = fpool.tile([128, QT], bf16, tag="scores_bf")
            nc.vector.tensor_copy(scores_bf[:], scores_sb[:, 0:QT])
            dscr = dpool.tile([HL, WL], bf16, tag="dscr")
            nc.sync.dma_start(
                dscr[:].rearrange("(t x) w -> (x w) t", t=QT), scores_bf[:]
            )
            st_sb = fpool.tile([32, 32], bf16, tag="st")  # st[w, h] = S[h, w]
            nc.sync.dma_start(st_sb[:], dscr[:].rearrange("h w -> w h"))

            # ---- bilinear upsample: out = R @ S @ R^T (bf16 inputs) ----
            psu = ppool.tile([128, GW], f32, tag="mega")
            nc.tensor.matmul(psu[:32, :OUT], st_sb[:], rt_sb[:],
                             start=True, stop=True)
            u_sb = fpool.tile([32, OUT], bf16, tag="u")
            nc.vector.tensor_copy(u_sb[:], psu[:32, :OUT])
            for i in range(4):
                po = ppool.tile([128, GW], f32, tag="mega")
                nc.tensor.matmul(po[:, :OUT], rt_sb[:, i * 128:(i + 1) * 128],
                                 u_sb[:], start=True, stop=True)
                osb = fpool.tile([128, OUT], f32, tag="osb")
                nc.vector.tensor_copy(osb[:], po[:, :OUT])
                nc.sync.dma_start(out_d[i * 128:(i + 1) * 128, :], osb[:])

    nc.compile()
    return nc


def _resize_matrix(n_in: int, n_out: int) -> np.ndarray:
    """Bilinear (half-pixel, edge-clamped) interpolation matrix [n_out, n_in].
    Matches jax.image.resize(method='bilinear') for upsampling."""
    R = np.zeros((n_out, n_in), dtype=np.float64)
    scale = n_in / n_out
    for i in range(n_out):
        src = (i + 0.5) * scale - 0.5
        a0 = int(np.floor(src))
        w = src - a0
        a0c = min(max(a0, 0), n_in - 1)
        a1c = min(max(a0 + 1, 0), n_in - 1)
        R[i, a0c] += 1.0 - w
        R[i, a1c] += w
    return R.astype(np.float32)


def _prep_inputs(embeddings: np.ndarray, bank: np.ndarray):
    """Host-side layout prep. Returns per-core input maps."""
    import ml_dtypes
    f = np.float32
    f8 = ml_dtypes.float8_e4m3fn
    emb = np.asarray(embeddings, dtype=f)
    bank = np.asarray(bank, dtype=f)

    # queries: [B, E, HL, WL] -> qT [B, E, Q] (E-major for the stationary side)
    qT = emb.reshape(B, E, Q)
    q2 = np.einsum("beq,beq->bq", qT, qT)               # [B, Q]
    qk8_all = (2.0 * qT).astype(f8).reshape(B, KC, 128, Q)
    qb_all = ((q2 + 768.0) / 9.0).reshape(B, QT, 128).transpose(0, 2, 1)

    bankP = np.zeros((NPAD, E), dtype=f)
    bankP[:N_BANK] = bank
    bank8 = np.ascontiguousarray(
        bankP.T.reshape(KC, 128, NG, GW).transpose(2, 0, 1, 3).astype(f8)
    )                                                   # [NG, KC, 128, GW]
    b2c = np.full(NPAD, np.nan, dtype=f)
    b2c[:N_BANK] = np.einsum("ne,ne->n", bank, bank) - 768.0
    c0 = b2c.astype(f8)
    c1 = (b2c - c0.astype(f)).astype(f8)
    c0[N_BANK:] = f8(224.0)
    c1[N_BANK:] = f8(224.0)
    aug = np.ascontiguousarray(
        np.stack([c0, c1], axis=0).reshape(2, NG, GW).transpose(1, 0, 2)
    )                                                   # [NG, 2, GW]

    bh = ml_dtypes.bfloat16
    rt = np.ascontiguousarray(_resize_matrix(HL, OUT).T.astype(bh))  # [32, 512]

    in_maps = [
        {
            "qk8": np.ascontiguousarray(qk8_all[b]),
            "qb": np.ascontiguousarray(qb_all[b].astype(f)),
            "bank8": bank8,
            "aug": aug,
            "rt": rt,
        }
        for b in range(B)
    ]
    return in_maps


def kernel(embeddings, bank, out_size, _trace=False, _trace_kwargs=None):
    from concourse import bass_utils

    assert int(out_size) == OUT
    if "nc" not in _CACHE:
        _CACHE["nc"] = _build_nc()
    nc = _CACHE["nc"]

    in_maps = _prep_inputs(np.asarray(embeddings), np.asarray(bank))
    res = bass_utils.run_bass_kernel_spmd(
        nc, in_maps, core_ids=list(range(B)), trace=_trace,
        **(_trace_kwargs or {}),
    )
    _CACHE["last_results"] = res
    out = np.stack([res.results[b]["out"] for b in range(B)])
    return out.reshape(B, 1, OUT, OUT).astype(np.float32)


# revision 15
# speedup vs baseline: 2.8500x; 1.1568x over previous
"""KNN anomaly-scoring kernel for Trainium2 (Bass/Tile), 8 NeuronCores.

Model: for each of B=8 images with a [768, 32, 32] embedding grid, compute the
mean Euclidean distance to the 3 nearest neighbors in a 20000x768 memory bank
per spatial location, then bilinear-upsample the 32x32 score map to 512x512.

Sharding: data-parallel over batch. Core b handles image b (1024 queries) with
a full bank replica. No collectives.

Per-core device program (v2 — fp8 DoubleRow + 4-engine top-k scan):
  - Ranking key on the TensorEngine in fp8e4 DoubleRow mode (2 k-tiles of 128
    per instruction, 0.5 cycles/output column):
      r[q, n] = 2q.b - (b2[n] - 768)
    via 3 DoubleRow matmuls over E=768 plus one k=1 DoubleRow "aug" matmul
    whose two pairs carry (b2-768) as a two-term fp8 expansion (coarse +
    residual), keeping b2 nearly exact. d2 = (q2 + 768) - r with q2 applied
    later as a per-partition activation bias.
  - Bank padded 20000 -> 20480 (pad entries get r = -448, never selected),
    10 groups of 2048 columns; PSUM as two [128, 2048] 4-bank tiles.
  - Top-k scan split across the three PSUM-capable/SBUF engines per chunk:
    ACT copies cols [0:1664] PSUM->SBUF, Pool folds them 2->1 with
    tensor_tensor(max), DVE max8's the folded 832 plus the remaining 384
    PSUM cols directly. 16 candidates per (query tile, group).
  - Final: DVE max8 over 160 candidates, ACT Sqrt(r*(-1/9) + (q2+768)/9)
    with accum_out -> mean of 3 neighbor distances.
  - Bilinear 32->512 upsample = R @ S @ R^T, two small f32r matmuls.
"""

import numpy as np

B, E, HL, WL = 8, 768, 32, 32
N_BANK = 20000
Q = HL * WL            # 1024 queries per image
QT = Q // 128          # 8 query tiles
KC = E // 128          # 6 contraction k-tiles of 128
KK = KC // 2           # 3 DoubleRow pairs
OUT = 512
NPAD = 20480
NG = 10                # bank column groups
GW = NPAD // NG        # 2048 columns per group
SC = GW // 512         # 4 matmul sub-chunks per group (one PSUM bank each)
WA = 1728              # ACT copy width per chunk (DVE folds 8->1 in bf16)
WTAIL = GW - WA        # DVE direct-psum max8 width (320)
NCAND = NG * 16        # 160 candidate slots per query

_CACHE = {}


def _build_nc():
    import concourse.bass as bass
    import concourse.bacc as bacc
    import concourse.mybir as mybir
    from concourse.tile import TileContext

    f32 = mybir.dt.float32
    f32r = mybir.dt.float32r
    f8 = mybir.dt.float8e4
    bf16 = mybir.dt.bfloat16
    DR = mybir.MatmulPerfMode.DoubleRow

    nc = bacc.Bacc("TRN2", target_bir_lowering=False, debug=False)

    qk8_d = nc.dram_tensor("qk8", [KC, 128, Q], f8, kind="ExternalInput")
    qb_d = nc.dram_tensor("qb", [128, QT], f32, kind="ExternalInput")
    bank8_d = nc.dram_tensor("bank8", [NG, KC, 128, GW], f8, kind="ExternalInput")
    aug_d = nc.dram_tensor("aug", [NG, 2, GW], f8, kind="ExternalInput")
    rt_d = nc.dram_tensor("rt", [32, OUT], bf16, kind="ExternalInput")
    out_d = nc.dram_tensor("out", [OUT, OUT], f32, kind="ExternalOutput")

    with TileContext(nc) as tc:
        with (
            tc.tile_pool(name="qpool", bufs=1) as qpool,
            tc.tile_pool(name="bpool", bufs=2) as bpool,
            tc.tile_pool(name="cpool", bufs=1) as cpool,
            tc.tile_pool(name="spool", bufs=3) as spool,
            tc.tile_pool(name="fpool", bufs=2) as fpool,
            tc.tile_pool(name="ppool", bufs=2, space=bass.MemorySpace.PSUM) as ppool,
            tc.tile_pool(name="dpool", bufs=1, space=bass.MemorySpace.DRAM) as dpool,
        ):
            # ---- persistent tiles; DMAs ordered so chunk 0 starts early ----
            qk_sb = [qpool.tile([128, 2, Q], f8, tag=f"qk{kk}", name=f"qk{kk}")
                     for kk in range(KK)]
            for j in range(2):
                nc.sync.dma_start(qk_sb[0][:, j, :], qk8_d[j])
            qb_sb = qpool.tile([128, QT], f32, tag="qb")
            rt_sb = qpool.tile([32, OUT], bf16, tag="rt")
            neg1_sb = qpool.tile([1, 2, 128], f8, tag="neg1")
            nc.vector.memset(neg1_sb[:], -1.0)

            cand = [
                cpool.tile([128, NCAND], f32, tag=f"cand{t}", name=f"cand{t}")
                for t in range(QT)
            ]
            scores_sb = cpool.tile([128, 32], f32, tag="scores")
            nc.vector.memset(scores_sb[:], 0.0)

            def emit_final(t):
                fin8 = fpool.tile([128, 8], f32, tag="fin8")
                nc.vector.max(fin8[:], cand[t][:])
                d3 = fpool.tile([128, 3], f32, tag="d3")
                nc.scalar.activation(
                    d3[:], fin8[:, 0:3], mybir.ActivationFunctionType.Sqrt,
                    scale=-1.0 / 9.0, bias=qb_sb[:, t:t + 1],
                    accum_out=scores_sb[:, t:t + 1],
                )

            def emit_folds(cp, g, t):
                # bf16 fold chain 1728 -> 864 -> 432 -> 216, then top-8.
                f1 = spool.tile([128, WA // 2], bf16, tag="f1")
                nc.vector.tensor_tensor(
                    f1[:], cp[:, 0:WA // 2], cp[:, WA // 2:WA],
                    op=mybir.AluOpType.max,
                )
                f2 = spool.tile([128, WA // 4], bf16, tag="f2")
                nc.vector.tensor_tensor(
                    f2[:], f1[:, 0:WA // 4], f1[:, WA // 4:WA // 2],
                    op=mybir.AluOpType.max,
                )
                f3 = spool.tile([128, WA // 8], bf16, tag="f3")
                nc.vector.tensor_tensor(
                    f3[:], f2[:, 0:WA // 8], f2[:, WA // 8:WA // 4],
                    op=mybir.AluOpType.max,
                )
                nc.vector.max(cand[t][:, g * 16:g * 16 + 8], f3[:])
                if g == NG - 1:
                    emit_final(t)

            # ---- ranking key + per-chunk top-16 (1-chunk fold pipeline) ----
            prev = None
            for g in range(NG):
                bk = [bpool.tile([128, 2, GW], f8, tag=f"bank{kk}", name=f"bk{kk}")
                      for kk in range(KK)]
                au = bpool.tile([1, 2, GW], f8, tag="aug")
                if g == 0:
                    # interleave so the first sub-chunk's inputs arrive first
                    for j in range(2):
                        nc.sync.dma_start(bk[0][:, j, :], bank8_d[g, j])
                    nc.sync.dma_start(au[:], aug_d[g])
                    for kk in range(1, KK):
                        for j in range(2):
                            nc.sync.dma_start(qk_sb[kk][:, j, :],
                                              qk8_d[2 * kk + j])
                        for j in range(2):
                            nc.sync.dma_start(bk[kk][:, j, :],
                                              bank8_d[g, 2 * kk + j])
                    nc.sync.dma_start(qb_sb[:, 0:QT], qb_d[:])
                    nc.sync.dma_start(rt_sb[:], rt_d[:])
                else:
                    for kk in range(KK):
                        for j in range(2):
                            nc.sync.dma_start(bk[kk][:, j, :],
                                              bank8_d[g, 2 * kk + j])
                    nc.sync.dma_start(au[:], aug_d[g])
                for t in range(QT):
                    # Two 2-bank PSUM tiles per chunk: psA (sub-chunks 0-1)
                    # drains via copy_a mid-chunk, decoupling the PSUM ring
                    # from the PE critical path.
                    psA = ppool.tile([128, GW // 2], f32, tag="megaA")
                    psB = ppool.tile([128, GW // 2], f32, tag="megaB")
                    cp = spool.tile([128, WA], bf16, tag="cp")
                    for c in range(SC):
                        ps = psA if c < 2 else psB
                        dst = ps[:, (c % 2) * 512:(c % 2 + 1) * 512]
                        for kk in range(KK):
                            nc.tensor.matmul(
                                dst,
                                qk_sb[kk][:, :, t * 128:(t + 1) * 128],
                                bk[kk][:, :, c * 512:(c + 1) * 512],
                                start=(kk == 0), stop=False, perf_mode=DR,
                            )
                        nc.tensor.matmul(
                            dst,
                            neg1_sb[:],
                            au[:, :, c * 512:(c + 1) * 512],
                            start=False, stop=True, perf_mode=DR,
                        )
                        if c == 1:
                            nc.scalar.copy(cp[:, 0:1024], psA[:])
                    nc.scalar.copy(cp[:, 1024:WA], psB[:, 0:WA - 1024])
                    nc.vector.max(cand[t][:, g * 16 + 8:g * 16 + 16],
                                  psB[:, WA - 1024:])
                    # Fold chain for the PREVIOUS chunk (its copy is done by
                    # now) so DVE never head-of-line blocks on a fresh copy.
                    if prev is not None:
                        emit_folds(*prev)
                    prev = (cp, g, t)
            emit_folds(*prev)

            # ---- re-layout scores [128, 8] -> S [32, 32] via DRAM bounce ----
            # query index q = t*128 + p; (h, w) = (q // 32, q % 32)
            scores_bf = fpool.tile([128, QT], bf16, tag="scores_bf")
            nc.vector.tensor_copy(scores_bf[:], scores_sb[:, 0:QT])
            dscr = dpool.tile([HL, WL], bf16, tag="dscr")
            nc.sync.dma_start(
                dscr[:].rearrange("(t x) w -> (x w) t", t=QT), scores_bf[:]
            )
            st_sb = fpool.tile([32, 32], bf16, tag="st")  # st[w, h] = S[h, w]
            nc.sync.dma_start(st_sb[:], dscr[:].rearrange("h w -> w h"))

            # ---- bilinear upsample: out = R @ S @ R^T (bf16 inputs) ----
            psu = ppool.tile([128, GW // 2], f32, tag="megaA")
            nc.tensor.matmul(psu[:32, :OUT], st_sb[:], rt_sb[:],
                             start=True, stop=True)
            u_sb = fpool.tile([32, OUT], bf16, tag="u")
            nc.vector.tensor_copy(u_sb[:], psu[:32, :OUT])
            for i in range(4):
                po = ppool.tile([128, GW // 2], f32, tag="megaB")
                nc.tensor.matmul(po[:, :OUT], rt_sb[:, i * 128:(i + 1) * 128],
                                 u_sb[:], start=True, stop=True)
                osb = fpool.tile([128, OUT], f32, tag="osb")
                nc.vector.tensor_copy(osb[:], po[:, :OUT])
                nc.sync.dma_start(out_d[i * 128:(i + 1) * 128, :], osb[:])

    nc.compile()
    return nc


def _resize_matrix(n_in: int, n_out: int) -> np.ndarray:
    """Bilinear (half-pixel, edge-clamped) interpolation matrix [n_out, n_in].
    Matches jax.image.resize(method='bilinear') for upsampling."""
    R = np.zeros((n_out, n_in), dtype=np.float64)
    scale = n_in / n_out
    for i in range(n_out):
        src = (i + 0.5) * scale - 0.5
        a0 = int(np.floor(src))
        w = src - a0
        a0c = min(max(a0, 0), n_in - 1)
        a1c = min(max(a0 + 1, 0), n_in - 1)
        R[i, a0c] += 1.0 - w
        R[i, a1c] += w
    return R.astype(np.float32)


def _prep_inputs(embeddings: np.ndarray, bank: np.ndarray):
    """Host-side layout prep. Returns per-core input maps."""
    import ml_dtypes
    f = np.float32
    f8 = ml_dtypes.float8_e4m3fn
    emb = np.asarray(embeddings, dtype=f)
    bank = np.asarray(bank, dtype=f)

    # queries: [B, E, HL, WL] -> qT [B, E, Q] (E-major for the stationary side)
    qT = emb.reshape(B, E, Q)
    q2 = np.einsum("beq,beq->bq", qT, qT)               # [B, Q]
    qk8_all = (2.0 * qT).astype(f8).reshape(B, KC, 128, Q)
    qb_all = ((q2 + 768.0) / 9.0).reshape(B, QT, 128).transpose(0, 2, 1)

    bankP = np.zeros((NPAD, E), dtype=f)
    bankP[:N_BANK] = bank
    bank8 = np.ascontiguousarray(
        bankP.T.reshape(KC, 128, NG, GW).transpose(2, 0, 1, 3).astype(f8)
    )                                                   # [NG, KC, 128, GW]
    b2c = np.full(NPAD, np.nan, dtype=f)
    b2c[:N_BANK] = np.einsum("ne,ne->n", bank, bank) - 768.0
    c0 = b2c.astype(f8)
    c1 = (b2c - c0.astype(f)).astype(f8)
    c0[N_BANK:] = f8(224.0)
    c1[N_BANK:] = f8(224.0)
    aug = np.ascontiguousarray(
        np.stack([c0, c1], axis=0).reshape(2, NG, GW).transpose(1, 0, 2)
    )                                                   # [NG, 2, GW]

    bh = ml_dtypes.bfloat16
    rt = np.ascontiguousarray(_resize_matrix(HL, OUT).T.astype(bh))  # [32, 512]

    in_maps = [
        {
            "qk8": np.ascontiguousarray(qk8_all[b]),
            "qb": np.ascontiguousarray(qb_all[b].astype(f)),
            "bank8": bank8,
            "aug": aug,
            "rt": rt,
        }
        for b in range(B)
    ]
    return in_maps


def kernel(embeddings, bank, out_size, _trace=False, _trace_kwargs=None):
    from concourse import bass_utils

    assert int(out_size) == OUT
    if "nc" not in _CACHE:
        _CACHE["nc"] = _build_nc()
    nc = _CACHE["nc"]

    in_maps = _prep_inputs(np.asarray(embeddings), np.asarray(bank))
    res = bass_utils.run_bass_kernel_spmd(
        nc, in_maps, core_ids=list(range(B)), trace=_trace,
        **(_trace_kwargs or {}),
    )
    _CACHE["last_results"] = res
    out = np.stack([res.results[b]["out"] for b in range(B)])
    return out.reshape(B, 1, OUT, OUT).astype(np.float32)


# revision 17
# speedup vs baseline: 2.9819x; 1.0463x over previous
"""KNN anomaly-scoring kernel for Trainium2 (Bass/Tile), 8 NeuronCores.

Model: for each of B=8 images with a [768, 32, 32] embedding grid, compute the
mean Euclidean distance to the 3 nearest neighbors in a 20000x768 memory bank
per spatial location, then bilinear-upsample the 32x32 score map to 512x512.

Sharding: data-parallel over batch. Core b handles image b (1024 queries) with
a full bank replica. No collectives.

Per-core device program (v2 — fp8 DoubleRow + 4-engine top-k scan):
  - Ranking key on the TensorEngine in fp8e4 DoubleRow mode (2 k-tiles of 128
    per instruction, 0.5 cycles/output column):
      r[q, n] = 2q.b - (b2[n] - 768)
    via 3 DoubleRow matmuls over E=768 plus one k=1 DoubleRow "aug" matmul
    whose two pairs carry (b2-768) as a two-term fp8 expansion (coarse +
    residual), keeping b2 nearly exact. d2 = (q2 + 768) - r with q2 applied
    later as a per-partition activation bias.
  - Bank padded 20000 -> 20480 (pad entries get r = -448, never selected),
    10 groups of 2048 columns; PSUM as two [128, 2048] 4-bank tiles.
  - Top-k scan split across the three PSUM-capable/SBUF engines per chunk:
    ACT copies cols [0:1664] PSUM->SBUF, Pool folds them 2->1 with
    tensor_tensor(max), DVE max8's the folded 832 plus the remaining 384
    PSUM cols directly. 16 candidates per (query tile, group).
  - Final: DVE max8 over 160 candidates, ACT Sqrt(r*(-1/9) + (q2+768)/9)
    with accum_out -> mean of 3 neighbor distances.
  - Bilinear 32->512 upsample = R @ S @ R^T, two small f32r matmuls.
"""

import numpy as np

B, E, HL, WL = 8, 768, 32, 32
N_BANK = 20000
Q = HL * WL            # 1024 queries per image
QT = Q // 128          # 8 query tiles
KC = E // 128          # 6 contraction k-tiles of 128
KK = KC // 2           # 3 DoubleRow pairs
OUT = 512
NPAD = 20480
NG = 10                # bank column groups
GW = NPAD // NG        # 2048 columns per group
SC = GW // 512         # 4 matmul sub-chunks per group (one PSUM bank each)
WA = 1728              # ACT copy width per chunk (DVE folds 8->1 in bf16)
WTAIL = GW - WA        # DVE direct-psum max8 width (320)
NCAND = NG * 16        # 160 candidate slots per query

_CACHE = {}


def _build_nc():
    import concourse.bass as bass
    import concourse.bacc as bacc
    import concourse.mybir as mybir
    from concourse.tile import TileContext

    f32 = mybir.dt.float32
    f32r = mybir.dt.float32r
    f8 = mybir.dt.float8e4
    bf16 = mybir.dt.bfloat16
    DR = mybir.MatmulPerfMode.DoubleRow

    nc = bacc.Bacc("TRN2", target_bir_lowering=False, debug=False)

    qk8_d = nc.dram_tensor("qk8", [128, KC, Q], f8, kind="ExternalInput")
    qb_d = nc.dram_tensor("qb", [128, QT], f32, kind="ExternalInput")
    bank8_d = nc.dram_tensor("bank8", [NG, KC, 128, GW], f8, kind="ExternalInput")
    aug_d = nc.dram_tensor("aug", [NG, 2, GW], f8, kind="ExternalInput")
    rt_d = nc.dram_tensor("rt", [32, OUT], bf16, kind="ExternalInput")
    rtp4_d = nc.dram_tensor("rtp4", [128, OUT], bf16, kind="ExternalInput")
    out_d = nc.dram_tensor("out", [OUT, OUT], f32, kind="ExternalOutput")

    with TileContext(nc) as tc:
        with (
            tc.tile_pool(name="qpool", bufs=1) as qpool,
            tc.tile_pool(name="bpool", bufs=2) as bpool,
            tc.tile_pool(name="cpool", bufs=1) as cpool,
            tc.tile_pool(name="spool", bufs=3) as spool,
            tc.tile_pool(name="fpool", bufs=2) as fpool,
            tc.tile_pool(name="ppool", bufs=2, space=bass.MemorySpace.PSUM) as ppool,
            tc.tile_pool(name="dpool", bufs=1, space=bass.MemorySpace.DRAM) as dpool,
        ):
            # ---- persistent tiles; DMAs ordered so chunk 0 starts early ----
            qk_all = qpool.tile([128, KC, Q], f8, tag="qk")
            nc.sync.dma_start(qk_all[:], qk8_d[:])
            qk_sb = [qk_all[:, 2 * kk:2 * kk + 2, :] for kk in range(KK)]
            qb_sb = qpool.tile([128, QT], f32, tag="qb")
            rt_sb = qpool.tile([32, OUT], bf16, tag="rt")
            rtp4_sb = qpool.tile([128, OUT], bf16, tag="rtp4")
            neg1_sb = qpool.tile([1, 2, 128], f8, tag="neg1")
            nc.vector.memset(neg1_sb[:], -1.0)

            cand = [
                cpool.tile([128, NCAND], f32, tag=f"cand{t}", name=f"cand{t}")
                for t in range(QT)
            ]
            scores_sb = cpool.tile([128, 32], f32, tag="scores")
            nc.vector.memset(scores_sb[:], 0.0)

            def emit_final(t):
                fin8 = fpool.tile([128, 8], f32, tag="fin8")
                nc.vector.max(fin8[:], cand[t][:])
                d3 = fpool.tile([128, 3], f32, tag="d3")
                nc.scalar.activation(
                    d3[:], fin8[:, 0:3], mybir.ActivationFunctionType.Sqrt,
                    scale=-1.0 / 9.0, bias=qb_sb[:, t:t + 1],
                    accum_out=scores_sb[:, t:t + 1],
                )

            def emit_folds(cp, g, t):
                # bf16 fold chain 1728 -> 864 -> 432 -> 216, then top-8.
                f1 = spool.tile([128, WA // 2], bf16, tag="f1")
                nc.vector.tensor_tensor(
                    f1[:], cp[:, 0:WA // 2], cp[:, WA // 2:WA],
                    op=mybir.AluOpType.max,
                )
                f2 = spool.tile([128, WA // 4], bf16, tag="f2")
                nc.vector.tensor_tensor(
                    f2[:], f1[:, 0:WA // 4], f1[:, WA // 4:WA // 2],
                    op=mybir.AluOpType.max,
                )
                f3 = spool.tile([128, WA // 8], bf16, tag="f3")
                nc.vector.tensor_tensor(
                    f3[:], f2[:, 0:WA // 8], f2[:, WA // 8:WA // 4],
                    op=mybir.AluOpType.max,
                )
                nc.vector.max(cand[t][:, g * 16:g * 16 + 8], f3[:])
                if g == NG - 1:
                    emit_final(t)

            # ---- ranking key + per-chunk top-16 (1-chunk fold pipeline) ----
            prev = None
            for g in range(NG):
                bk = [bpool.tile([128, 2, GW], f8, tag=f"bank{kk}", name=f"bk{kk}")
                      for kk in range(KK)]
                au = bpool.tile([1, 2, GW], f8, tag="aug")
                if g == 0:
                    # interleave so the first sub-chunk's inputs arrive first
                    for j in range(2):
                        nc.sync.dma_start(bk[0][:, j, :], bank8_d[g, j])
                    nc.sync.dma_start(au[:], aug_d[g])
                    for kk in range(1, KK):
                        for j in range(2):
                            nc.sync.dma_start(bk[kk][:, j, :],
                                              bank8_d[g, 2 * kk + j])
                    nc.sync.dma_start(qb_sb[:, 0:QT], qb_d[:])
                    nc.sync.dma_start(rt_sb[:], rt_d[:])
                    nc.sync.dma_start(rtp4_sb[:], rtp4_d[:])
                else:
                    for kk in range(KK):
                        for j in range(2):
                            nc.sync.dma_start(bk[kk][:, j, :],
                                              bank8_d[g, 2 * kk + j])
                    nc.sync.dma_start(au[:], aug_d[g])
                for t in range(QT):
                    # Two 2-bank PSUM tiles per chunk: psA (sub-chunks 0-1)
                    # drains via copy_a mid-chunk, decoupling the PSUM ring
                    # from the PE critical path.
                    psA = ppool.tile([128, GW // 2], f32, tag="megaA")
                    psB = ppool.tile([128, GW // 2], f32, tag="megaB")
                    cp = spool.tile([128, WA], bf16, tag="cp")
                    for c in range(SC):
                        ps = psA if c < 2 else psB
                        dst = ps[:, (c % 2) * 512:(c % 2 + 1) * 512]
                        for kk in range(KK):
                            nc.tensor.matmul(
                                dst,
                                qk_all[:, 2 * kk:2 * kk + 2,
                                       t * 128:(t + 1) * 128],
                                bk[kk][:, :, c * 512:(c + 1) * 512],
                                start=(kk == 0), stop=False, perf_mode=DR,
                            )
                        nc.tensor.matmul(
                            dst,
                            neg1_sb[:],
                            au[:, :, c * 512:(c + 1) * 512],
                            start=False, stop=True, perf_mode=DR,
                        )
                        if c == 1:
                            nc.scalar.copy(cp[:, 0:1024], psA[:])
                    nc.scalar.copy(cp[:, 1024:WA], psB[:, 0:WA - 1024])
                    nc.vector.max(cand[t][:, g * 16 + 8:g * 16 + 16],
                                  psB[:, WA - 1024:])
                    # Fold chain for the PREVIOUS chunk (its copy is done by
                    # now) so DVE never head-of-line blocks on a fresh copy.
                    if prev is not None:
                        emit_folds(*prev)
                    prev = (cp, g, t)
            emit_folds(*prev)

            # ---- scores -> S via in-place DVE block transposes ----
            # Query q = t*128 + p, (h, w) = (q//32, q%32): score for S[h, w]
            # sits at scores_sb[32j + w, t] with h = 4t + j. Transposing each
            # 32x32 block in place puts S[4t+j, w] at partition 32j + t, col
            # w. Stage 1 contracts the scattered S rows with rtp4 (host-
            # replicated R^T rows at matching partitions) via 4 accumulating
            # K=8 matmuls.
            scores_bf = fpool.tile([128, 32], bf16, tag="scores_bf")
            nc.vector.tensor_copy(scores_bf[:], scores_sb[:])
            trp = fpool.tile([128, 32], bf16, tag="trp")
            for j in range(4):
                nc.vector.transpose(trp[32 * j:32 * (j + 1), :],
                                    scores_bf[32 * j:32 * (j + 1), :])

            # ---- bilinear upsample: out = R @ S @ R^T (bf16 inputs) ----
            # Single K=128 contraction: partitions 32j+t (t<8) carry S rows;
            # all other partitions are zeros (memset) x zero rtp4 rows.
            psu = ppool.tile([128, GW // 2], f32, tag="megaA")
            nc.tensor.matmul(psu[:32, :OUT], trp[:], rtp4_sb[:],
                             start=True, stop=True)
            u_sb = fpool.tile([32, OUT], bf16, tag="u")
            nc.vector.tensor_copy(u_sb[:], psu[:32, :OUT])
            for i in range(4):
                po = ppool.tile([128, GW // 2], f32, tag="megaB")
                nc.tensor.matmul(po[:, :OUT], rt_sb[:, i * 128:(i + 1) * 128],
                                 u_sb[:], start=True, stop=True)
                osb = fpool.tile([128, OUT], f32, tag=f"osb{i % 2}")
                if i % 2 == 0:
                    nc.vector.tensor_copy(osb[:], po[:, :OUT])
                else:
                    nc.scalar.copy(osb[:], po[:, :OUT])
                nc.sync.dma_start(out_d[i * 128:(i + 1) * 128, :], osb[:])

    nc.compile()
    return nc


def _resize_matrix(n_in: int, n_out: int) -> np.ndarray:
    """Bilinear (half-pixel, edge-clamped) interpolation matrix [n_out, n_in].
    Matches jax.image.resize(method='bilinear') for upsampling."""
    R = np.zeros((n_out, n_in), dtype=np.float64)
    scale = n_in / n_out
    for i in range(n_out):
        src = (i + 0.5) * scale - 0.5
        a0 = int(np.floor(src))
        w = src - a0
        a0c = min(max(a0, 0), n_in - 1)
        a1c = min(max(a0 + 1, 0), n_in - 1)
        R[i, a0c] += 1.0 - w
        R[i, a1c] += w
    return R.astype(np.float32)


def _prep_inputs(embeddings: np.ndarray, bank: np.ndarray):
    """Host-side layout prep. Returns per-core input maps."""
    import ml_dtypes
    f = np.float32
    f8 = ml_dtypes.float8_e4m3fn
    emb = np.asarray(embeddings, dtype=f)
    bank = np.asarray(bank, dtype=f)

    # queries: [B, E, HL, WL] -> qT [B, E, Q] (E-major for the stationary side)
    qT = emb.reshape(B, E, Q)
    q2 = np.einsum("beq,beq->bq", qT, qT)               # [B, Q]
    qk8_all = np.ascontiguousarray(
        (2.0 * qT).astype(f8).reshape(B, KC, 128, Q).transpose(0, 2, 1, 3)
    )                                                   # [B, 128, KC, Q]
    qb_all = ((q2 + 768.0) / 9.0).reshape(B, QT, 128).transpose(0, 2, 1)

    bankP = np.zeros((NPAD, E), dtype=f)
    bankP[:N_BANK] = bank
    bank8 = np.ascontiguousarray(
        bankP.T.reshape(KC, 128, NG, GW).transpose(2, 0, 1, 3).astype(f8)
    )                                                   # [NG, KC, 128, GW]
    b2c = np.full(NPAD, np.nan, dtype=f)
    b2c[:N_BANK] = np.einsum("ne,ne->n", bank, bank) - 768.0
    c0 = b2c.astype(f8)
    c1 = (b2c - c0.astype(f)).astype(f8)
    c0[N_BANK:] = f8(224.0)
    c1[N_BANK:] = f8(224.0)
    aug = np.ascontiguousarray(
        np.stack([c0, c1], axis=0).reshape(2, NG, GW).transpose(1, 0, 2)
    )                                                   # [NG, 2, GW]

    bh = ml_dtypes.bfloat16
    rt = np.ascontiguousarray(_resize_matrix(HL, OUT).T.astype(bh))  # [32, 512]
    rtp4 = np.zeros((128, OUT), dtype=bh)
    for j in range(4):
        for t in range(8):
            rtp4[32 * j + t] = rt[4 * t + j]

    in_maps = [
        {
            "qk8": np.ascontiguousarray(qk8_all[b]),
            "qb": np.ascontiguousarray(qb_all[b].astype(f)),
            "bank8": bank8,
            "aug": aug,
            "rt": rt,
            "rtp4": rtp4,
        }
        for b in range(B)
    ]
    return in_maps


def kernel(embeddings, bank, out_size, _trace=False, _trace_kwargs=None):
    from concourse import bass_utils

    assert int(out_size) == OUT
    if "nc" not in _CACHE:
        _CACHE["nc"] = _build_nc()
    nc = _CACHE["nc"]

    in_maps = _prep_inputs(np.asarray(embeddings), np.asarray(bank))
    res = bass_utils.run_bass_kernel_spmd(
        nc, in_maps, core_ids=list(range(B)), trace=_trace,
        **(_trace_kwargs or {}),
    )
    _CACHE["last_results"] = res
    out = np.stack([res.results[b]["out"] for b in range(B)])
    return out.reshape(B, 1, OUT, OUT).astype(np.float32)
